# revision 1
# baseline (speedup 1.0000x reference)
"""Trainium2 Bass kernel for nn_AverageAttn (B=4, S=4096, D=H=1024, 8 cores).

out = igate * iQ + fgate * h, where
  avg  = causal cumulative average of iV along seq
  h    = relu(avg @ W1 + b1) @ W2 + b2
  ifg  = sigmoid(concat(iQ, h) @ Wg + bg);  igate, fgate = split(ifg)

Sharding: 8 cores = (batch b, seq half h).  Each core processes 2048 tokens.
Cores with h=1 also stream iV[b, :2048] to build the prefix chunk-sums.

On-device layout is "T-orientation": activations live as [feature, token]
tiles so matmuls chain without transposes; only iQ (in) and out (back) cross
orientation via PE transposes.  All matmul operands are float32r (TF32-like,
1 cycle/row at N>=256).
"""

import numpy as np

B, S, D = 4, 4096, 1024
H = 1024
T = S // 2              # tokens per core
P = 128
NCH = T // P            # 16 chunks of 128 tokens per core
NBLK = 4                # 512-token blocks per core
CPB = 4                 # chunks per block
ND = D // P             # 8 feature chunks
NG = 2 * D // P         # 16 gate chunks
NROW = 32               # S-table rows: 0..15 prefix, 16..31 shard chunks


def _host_constants():
    """Per-parity constants: scaled triangular blocks and carry masks."""
    consts = {}
    for half in (0, 1):
        off = half * T
        # ltri[t, c, s] = 1/(off + 128c + s + 1) if t <= s else 0
        ltri = np.zeros((P, NCH, P), np.float32)
        t = np.arange(P)[:, None]
        s = np.arange(P)[None, :]
        for c in range(NCH):
            denom = 1.0 / (off + P * c + s + 1).astype(np.float32)
            ltri[:, c, :] = np.where(t <= s, denom, 0.0)
        # mask[r, b, s] = 1/(off + 512b + s + 1) if S-row r feeds chunk of s
        mask = np.zeros((P, NBLK, 4 * P), np.float32)
        sb = np.arange(4 * P)
        for b in range(NBLK):
            w = 1.0 / (off + 4 * P * b + sb + 1).astype(np.float32)
            cc = sb // P  # chunk-in-block of each s
            for r in range(NROW):
                if r < 16:
                    inc = np.full(4 * P, half == 1)
                else:
                    inc = (r - 16) < (4 * b + cc)
                mask[r, b, :] = np.where(inc, w, 0.0)
        ltri_b = np.ascontiguousarray(
            ltri.reshape(P, NBLK, CPB, P).transpose(1, 0, 2, 3))
        mask_b = np.ascontiguousarray(mask.transpose(1, 0, 2))
        consts[half] = (ltri_b, mask_b)
    # oband[p, i] = 1 iff i == 32  ->  lhsT for S-row r is oband[:, 32-r:160-r]
    oband = np.zeros((P, 160), np.float32)
    oband[:, 32] = 1.0
    ident = np.eye(P, dtype=np.float32)
    return consts, oband, ident


def _build_program():
    import concourse.bass as bass  # noqa: F401
    import concourse.tile as tile
    from concourse import mybir, bacc

    f32 = mybir.dt.float32
    f32r = mybir.dt.float32r
    Relu = mybir.ActivationFunctionType.Relu
    Ident = mybir.ActivationFunctionType.Identity
    Sigm = mybir.ActivationFunctionType.Sigmoid

    nc = bacc.Bacc("TRN2", target_bir_lowering=False)

    q = nc.dram_tensor("q", [NCH, P, D], f32, kind="ExternalInput")
    v = nc.dram_tensor("v", [NCH, P, D], f32, kind="ExternalInput")
    vpre = nc.dram_tensor("vpre", [NCH, P, D], f32, kind="ExternalInput")
    w1s = nc.dram_tensor("w1s", [ND, P, ND, P], f32, kind="ExternalInput")
    w2s = nc.dram_tensor("w2s", [ND, P, ND, P], f32, kind="ExternalInput")
    wgs = nc.dram_tensor("wgs", [NG, P, NG, P], f32, kind="ExternalInput")
    b1c = nc.dram_tensor("b1c", [P, ND], f32, kind="ExternalInput")
    b2c = nc.dram_tensor("b2c", [P, ND], f32, kind="ExternalInput")
    bgc = nc.dram_tensor("bgc", [P, NG], f32, kind="ExternalInput")
    ltri = nc.dram_tensor("ltri", [NBLK, P, CPB, P], f32, kind="ExternalInput")
    maskd = nc.dram_tensor("maskd", [NBLK, P, 4 * P], f32, kind="ExternalInput")
    oband = nc.dram_tensor("oband", [P, 160], f32, kind="ExternalInput")
    ident = nc.dram_tensor("ident", [P, P], f32, kind="ExternalInput")
    o = nc.dram_tensor("o", [NCH, P, D], f32, kind="ExternalOutput")

    with tile.TileContext(nc) as tc:
        import contextlib
        ctx = contextlib.ExitStack()
        with ctx:
            cpool = ctx.enter_context(tc.tile_pool(name="consts", bufs=1))
            vpool = ctx.enter_context(tc.tile_pool(name="vq", bufs=4))
            qpool = ctx.enter_context(tc.tile_pool(name="qp", bufs=2))
            wpool = ctx.enter_context(tc.tile_pool(name="wslab", bufs=2))
            apool = ctx.enter_context(tc.tile_pool(name="acts", bufs=2))
            a1pool = ctx.enter_context(tc.tile_pool(name="acts1", bufs=1))
            avpool = ctx.enter_context(tc.tile_pool(name="avsl", bufs=16))
            spool = ctx.enter_context(tc.tile_pool(name="small", bufs=2))
            opool = ctx.enter_context(tc.tile_pool(name="outs", bufs=1))
            ps_mm = ctx.enter_context(tc.tile_pool(name="psmm", bufs=2, space="PSUM"))
            ps_cum = ctx.enter_context(tc.tile_pool(name="pscum", bufs=2, space="PSUM"))
            ps_tr = ctx.enter_context(tc.tile_pool(name="pstr", bufs=2, space="PSUM"))
            ps_sp = ctx.enter_context(tc.tile_pool(name="pssp", bufs=1, space="PSUM"))

            # ---- constants -------------------------------------------------
            identT = cpool.tile([P, P], f32r, tag="ident")
            nc.sync.dma_start(identT[:], ident[:].bitcast(f32r))
            obandT = cpool.tile([P, 160], f32r, tag="oband")
            nc.sync.dma_start(obandT[:], oband[:].bitcast(f32r))
            b1T = cpool.tile([P, ND], f32, tag="b1")
            nc.sync.dma_start(b1T[:], b1c[:])
            b2T = cpool.tile([P, ND], f32, tag="b2")
            nc.sync.dma_start(b2T[:], b2c[:])
            bgT = cpool.tile([P, NG], f32, tag="bg")
            nc.sync.dma_start(bgT[:], bgc[:])

            S_sb = cpool.tile([P, D], f32r, tag="Ssb")

            def srow_lhsT(r):
                return obandT[:, 32 - r:160 - r]

            # ---- prefix pass: S rows 0..15 from vpre ----------------------
            sp = ps_sp.tile([P, D], f32, tag="sp")
            for c in range(NCH):
                vch = vpool.tile([P, D], f32r, tag="vch")
                nc.sync.dma_start(vch[:], vpre[c].bitcast(f32r))
                for hf in range(2):
                    nc.tensor.matmul(
                        sp[:, hf * 512:(hf + 1) * 512],
                        srow_lhsT(c),
                        vch[:, hf * 512:(hf + 1) * 512],
                        start=(c == 0), stop=(c == NCH - 1),
                        skip_group_check=True,
                    )
            nc.vector.tensor_copy(S_sb[:], sp[:])

            # ---- main: 2 pairs of 512-token blocks -------------------------
            iqT = {}
            hT = {}
            ig_sb = {}
            outT = {}
            avgT = {}
            h1T = {}

            streams = {}

            def stream_block(blk):
                vchs, qchs = [], []
                for cc in range(CPB):
                    c = blk * CPB + cc
                    vch = vpool.tile([P, D], f32r, tag="vch")
                    nc.sync.dma_start(vch[:], v[c].bitcast(f32r))
                    vchs.append(vch)
                    qch = qpool.tile([P, D], f32r, tag="qch")
                    nc.sync.dma_start(qch[:], q[c].bitcast(f32r))
                    qchs.append(qch)
                streams[blk] = (vchs, qchs)

            def scan_block(blk):
                """S-rows, iQ transposes, cumulative average."""
                vchs, qchs = streams.pop(blk)
                ltb = wpool.tile([P, CPB, P], f32r, tag="ltri")
                nc.sync.dma_start(ltb[:], ltri[blk].bitcast(f32r))
                mkb = wpool.tile([P, 4 * P], f32r, tag="mask")
                nc.sync.dma_start(mkb[:], maskd[blk].bitcast(f32r))

                sp = ps_sp.tile([P, D], f32, tag="sp")
                for cc in range(CPB):
                    r = 16 + blk * CPB + cc
                    for hf in range(2):
                        nc.tensor.matmul(
                            sp[:, hf * 512:(hf + 1) * 512],
                            srow_lhsT(r),
                            vchs[cc][:, hf * 512:(hf + 1) * 512],
                            start=(cc == 0), stop=(cc == CPB - 1),
                            skip_group_check=True,
                        )
                nc.vector.tensor_add(S_sb[:], S_sb[:], sp[:])

                iqT[blk] = apool.tile([P, ND, 4 * P], f32r, tag="iqT", name="iqT")
                for tc in range(CPB):
                    for dh in range(2):
                        ptr = ps_tr.tile([P, 4 * P], f32r, tag="tr")
                        for dd in range(4):
                            d = dh * 4 + dd
                            nc.tensor.transpose(
                                ptr[:, dd * P:(dd + 1) * P],
                                qchs[tc][:, d * P:(d + 1) * P],
                                identT[:],
                            )
                        nc.vector.tensor_copy(
                            iqT[blk][:, dh * 4:(dh + 1) * 4, tc * P:(tc + 1) * P],
                            ptr[:].rearrange("p (a b) -> p a b", a=4))

                avgT[blk] = []
                for d in range(ND):
                    pav = ps_cum.tile([P, 4 * P], f32, tag="avg")
                    # cc=0 clears the whole bank (start=True); cc=1..3 land on
                    # has_written=0 slices (overwrite); carry accumulates last.
                    for cc in range(CPB):
                        nc.tensor.matmul(
                            pav[:, cc * P:(cc + 1) * P],
                            vchs[cc][:, d * P:(d + 1) * P],
                            ltb[:, cc, :],
                            start=(cc == 0), stop=False,
                            skip_group_check=True,
                        )
                    nc.tensor.matmul(
                        pav[:],
                        S_sb[:, d * P:(d + 1) * P],
                        mkb[:],
                        start=False, stop=True,
                        skip_group_check=True,
                    )
                    avsl = avpool.tile([P, 4 * P], f32r, tag="avgT",
                                       name="avsl")
                    nc.scalar.copy(avsl[:], pav[:])
                    avgT[blk].append(avsl)

            def ffn1_pair(blocks):
                for blk in blocks:
                    h1T[blk] = apool.tile([P, ND, 4 * P], f32r, tag="h1T",
                                          name="h1T")
                for j in range(ND):
                    w1t = wpool.tile([P, ND, P], f32r, tag="w12")
                    nc.sync.dma_start(w1t[:], w1s[j].bitcast(f32r))
                    for blk in blocks:
                        pm = ps_mm.tile([P, 4 * P], f32, tag="mm")
                        for d in range(ND):
                            nc.tensor.matmul(
                                pm[:], w1t[:, d, :], avgT[blk][d][:],
                                start=(d == 0), stop=(d == ND - 1),
                            )
                        nc.scalar.activation(h1T[blk][:, j, :], pm[:], Relu,
                                             bias=b1T[:, j:j + 1])

            def ffn2_pair(blocks):
                for blk in blocks:
                    hT[blk] = apool.tile([P, ND, 4 * P], f32r, tag="hT",
                                         name="hT")
                for d2 in range(ND):
                    w2t = wpool.tile([P, ND, P], f32r, tag="w12")
                    nc.sync.dma_start(w2t[:], w2s[d2].bitcast(f32r))
                    for blk in blocks:
                        pm = ps_mm.tile([P, 4 * P], f32, tag="mm")
                        for j in range(ND):
                            nc.tensor.matmul(
                                pm[:], w2t[:, j, :], h1T[blk][:, j, :],
                                start=(j == 0), stop=(j == ND - 1),
                            )
                        nc.scalar.activation(hT[blk][:, d2, :], pm[:], Ident,
                                             bias=b2T[:, d2:d2 + 1])

            for pair in range(2):
                blkA, blkB = 2 * pair, 2 * pair + 1
                blocks = (blkA, blkB)
                stream_block(blkA)
                scan_block(blkA)
                stream_block(blkB)
                scan_block(blkB)
                ffn1_pair(blocks)
                ffn2_pair(blocks)

                for blk in blocks:
                    outT[blk] = apool.tile([P, ND, 4 * P], f32r, tag="h1T",
                                           name="outT")
                for gp in range(ND):
                    for gg in (gp, gp + ND):
                        wgt = wpool.tile([P, NG, P], f32r, tag="wg")
                        nc.sync.dma_start(wgt[:], wgs[gg].bitcast(f32r))
                        for blk in blocks:
                            pg = ps_mm.tile([P, 4 * P], f32, tag="mm")
                            for c in range(NG):
                                rhs = (iqT[blk][:, c, :] if c < ND
                                       else hT[blk][:, c - ND, :])
                                nc.tensor.matmul(
                                    pg[:], wgt[:, c, :], rhs,
                                    start=(c == 0), stop=(c == NG - 1),
                                )
                            gate = spool.tile([P, 4 * P], f32r,
                                              tag=("ig" if gg < ND else "fg"))
                            nc.scalar.activation(gate[:], pg[:], Sigm,
                                                 bias=bgT[:, gg:gg + 1])
                            if gg < ND:
                                ig_sb[blk] = gate
                            else:
                                # final elementwise + transpose-out this d-chunk
                                ot = outT[blk][:, gp, :]
                                tmp = spool.tile([P, 4 * P], f32r, tag="tmp")
                                nc.vector.tensor_mul(
                                    tmp[:], ig_sb[blk][:], iqT[blk][:, gp, :])
                                nc.vector.tensor_mul(
                                    ot, gate[:], hT[blk][:, gp, :])
                                nc.vector.tensor_add(ot, ot, tmp[:])
                                ptr = ps_tr.tile([P, 4 * P], f32r, tag="tr")
                                for tc in range(CPB):
                                    nc.tensor.transpose(
                                        ptr[:, tc * P:(tc + 1) * P],
                                        outT[blk][:, gp, tc * P:(tc + 1) * P],
                                        identT[:],
                                    )
                                otp = spool.tile([P, CPB, P], f32, tag="otp")
                                nc.vector.tensor_copy(
                                    otp[:], ptr[:].rearrange("p (a b) -> p a b", a=CPB))
                                nc.sync.dma_start(
                                    o[blk * CPB:(blk + 1) * CPB, :,
                                      gp * P:(gp + 1) * P].rearrange("c p d -> p c d"),
                                    otp[:])

    nc.finalize()
    return nc


_CACHED = {}
_last_result = None


def kernel(iQ, iV, W1, b1, W2, b2, Wg, bg):
    import sys
    if '/opt/trn_rl_repo' not in sys.path:
        sys.path.insert(0, '/opt/trn_rl_repo')
    from concourse.bass_utils import run_bass_kernel_spmd

    iQ = np.asarray(iQ, np.float32)
    iV = np.asarray(iV, np.float32)
    W1 = np.asarray(W1, np.float32)
    b1 = np.asarray(b1, np.float32)
    W2 = np.asarray(W2, np.float32)
    b2 = np.asarray(b2, np.float32)
    Wg = np.asarray(Wg, np.float32)
    bg = np.asarray(bg, np.float32)

    if 'nc' not in _CACHED:
        _CACHED['nc'] = _build_program()
    nc = _CACHED['nc']

    consts, oband, ident = _host_constants()

    # weight slabs: lhsT tiles, slab[m][p, k, q] = W[k*128+p, m*128+q]
    def slabs(W, n):
        return np.ascontiguousarray(
            W.reshape(n, P, n, P).transpose(2, 1, 0, 3))

    w1s = slabs(W1, ND)
    w2s = slabs(W2, ND)
    wgs = slabs(Wg, NG)
    b1c = np.ascontiguousarray(b1.reshape(ND, P).T)
    b2c = np.ascontiguousarray(b2.reshape(ND, P).T)
    bgc = np.ascontiguousarray(bg.reshape(NG, P).T)
    zpre = np.zeros((NCH, P, D), np.float32)

    in_maps = []
    for core in range(8):
        b, half = core // 2, core % 2
        ltri_h, mask_h = consts[half]
        in_maps.append({
            "q": np.ascontiguousarray(
                iQ[b, half * T:(half + 1) * T].reshape(NCH, P, D)),
            "v": np.ascontiguousarray(
                iV[b, half * T:(half + 1) * T].reshape(NCH, P, D)),
            "vpre": (np.ascontiguousarray(iV[b, :T].reshape(NCH, P, D))
                     if half == 1 else zpre),
            "w1s": w1s, "w2s": w2s, "wgs": wgs,
            "b1c": b1c, "b2c": b2c, "bgc": bgc,
            "ltri": ltri_h, "maskd": mask_h,
            "oband": oband, "ident": ident,
        })

    res = run_bass_kernel_spmd(nc, in_maps, core_ids=list(range(8)))
    global _last_result
    _last_result = res

    out = np.empty((B, S, D), np.float32)
    for core in range(8):
        b, half = core // 2, core % 2
        out[b, half * T:(half + 1) * T] = res.results[core]["o"].reshape(T, D)
    return out



# revision 3
# speedup vs baseline: 1.5444x; 1.5444x over previous
"""Trainium2 Bass kernel for nn_AverageAttn (B=4, S=4096, D=H=1024, 8 cores).

out = igate * iQ + fgate * h, where
  avg  = causal cumulative average of iV along seq
  h    = relu(avg @ W1 + b1) @ W2 + b2
  ifg  = sigmoid(concat(iQ, h) @ Wg + bg);  igate, fgate = split(ifg)

Sharding: 8 cores = (batch b, seq half). Each core processes T=2048 tokens.

v2 design (vs fp32r baseline):
 - All activations arrive pre-transposed from host in T-orientation
   [feature, token]; output leaves T-oriented (host transposes back).
   No PE transposes at all.
 - Cumulative sum runs on the Vector engine (tensor_tensor_scan), the
   first-half carry comes from an Activation-engine accumulate pass over
   the streamed first-half iV; avg = (scan + carry) * (1/n) in one
   scalar_tensor_tensor op.  The PE does zero scan work.
 - FFN1/FFN2 and the h-half of the gate run in fp8 e4m3 with
   MatmulPerfMode.DoubleRow (K=256/instr); the iQ-half of the gate and
   everything scan-side stays bf16.  PSUM accumulates fp32.
 - DMA split across 3 queues: SP (v/q/out), Activation (weights),
   Pool SWDGE (first-half v stream).
"""

import numpy as np

B, S, D = 4, 4096, 1024
H = 1024
T = S // 2              # tokens per core
P = 128
ND = D // P             # 8 feature chunks
NG = 2 * D // P         # 16 gate chunks
NP = ND // 2            # 4 DoubleRow K-pairs
NBLK = 4                # 512-token matmul blocks
BT = T // NBLK          # 512


def _build_program():
    import contextlib
    import concourse.bass as bass  # noqa: F401
    import concourse.tile as tile
    from concourse import mybir, bacc

    f32 = mybir.dt.float32
    bf16 = mybir.dt.bfloat16
    fp8 = mybir.dt.float8e4
    Relu = mybir.ActivationFunctionType.Relu
    Ident = mybir.ActivationFunctionType.Identity
    Sigm = mybir.ActivationFunctionType.Sigmoid
    DR = mybir.MatmulPerfMode.DoubleRow
    Add = mybir.AluOpType.add
    Mult = mybir.AluOpType.mult
    Bypass = mybir.AluOpType.bypass

    nc = bacc.Bacc("TRN2", target_bir_lowering=False)

    vTd = nc.dram_tensor("vTd", [ND, P, T], bf16, kind="ExternalInput")
    vpreT = nc.dram_tensor("vpreT", [ND, P, T], bf16, kind="ExternalInput")
    qTd = nc.dram_tensor("qTd", [ND, P, T], bf16, kind="ExternalInput")
    denr = nc.dram_tensor("denr", [P, T], bf16, kind="ExternalInput")
    w1s = nc.dram_tensor("w1s", [ND, P, NP, 2, P], fp8, kind="ExternalInput")
    w2s = nc.dram_tensor("w2s", [ND, P, NP, 2, P], fp8, kind="ExternalInput")
    wgt = nc.dram_tensor("wgt", [NG, P, ND, P], bf16, kind="ExternalInput")
    wgb = nc.dram_tensor("wgb", [NG, P, NP, 2, P], fp8, kind="ExternalInput")
    b1c = nc.dram_tensor("b1c", [P, ND], f32, kind="ExternalInput")
    b2c = nc.dram_tensor("b2c", [P, ND], f32, kind="ExternalInput")
    bgc = nc.dram_tensor("bgc", [P, NG], f32, kind="ExternalInput")
    o = nc.dram_tensor("o", [ND, P, T], bf16, kind="ExternalOutput")

    with tile.TileContext(nc) as tc:
        ctx = contextlib.ExitStack()
        with ctx:
            cpool = ctx.enter_context(tc.tile_pool(name="consts", bufs=1))
            vpool = ctx.enter_context(tc.tile_pool(name="vstream", bufs=4))
            prepool = ctx.enter_context(tc.tile_pool(name="prestream", bufs=3))
            scanpool = ctx.enter_context(tc.tile_pool(name="scans", bufs=2))
            scrpool = ctx.enter_context(tc.tile_pool(name="scratch", bufs=2))
            vsumpool = ctx.enter_context(tc.tile_pool(name="vsum", bufs=ND))
            avgpool = ctx.enter_context(tc.tile_pool(name="avg", bufs=NP))
            h1pool = ctx.enter_context(tc.tile_pool(name="h1", bufs=NP))
            hpool = ctx.enter_context(tc.tile_pool(name="hh", bufs=NP))
            qpool = ctx.enter_context(tc.tile_pool(name="qq", bufs=1))
            wpool = ctx.enter_context(tc.tile_pool(name="w12", bufs=2))
            gwpool = ctx.enter_context(tc.tile_pool(name="gw", bufs=3))
            gatepool = ctx.enter_context(tc.tile_pool(name="gates", bufs=4))
            ewpool = ctx.enter_context(tc.tile_pool(name="ew", bufs=2))
            opool = ctx.enter_context(tc.tile_pool(name="outs", bufs=3))
            pspool = ctx.enter_context(
                tc.tile_pool(name="psmm", bufs=8, space="PSUM"))

            # ---- constants (Activation HWDGE queue) -----------------------
            denrT = cpool.tile([P, T], bf16, tag="denr")
            nc.scalar.dma_start(denrT[:], denr[:])
            b1T = cpool.tile([P, ND], f32, tag="b1")
            nc.scalar.dma_start(b1T[:], b1c[:])
            b2T = cpool.tile([P, ND], f32, tag="b2")
            nc.scalar.dma_start(b2T[:], b2c[:])
            bgT = cpool.tile([P, NG], f32, tag="bg")
            nc.scalar.dma_start(bgT[:], bgc[:])

            # ---- scan phase: avg8 pair tiles [P, 2, T] fp8 ----------------
            avg8 = [avgpool.tile([P, 2, T], fp8, tag="avg", name="avg8") for _ in range(NP)]
            for d in range(ND):
                vt = vpool.tile([P, T], bf16, tag="v")
                nc.sync.dma_start(vt[:], vTd[d])
                pre = prepool.tile([P, T], bf16, tag="pre")
                nc.gpsimd.dma_start(pre[:], vpreT[d])
                vsum = vsumpool.tile([P, 1], f32, tag="vsum")
                scr = scrpool.tile([P, T], bf16, tag="scr")
                nc.scalar.activation(scr[:], pre[:], Ident, accum_out=vsum[:])
                sc = scanpool.tile([P, T], bf16, tag="scan")
                nc.vector.tensor_tensor_scan(sc[:], vt[:], vt[:], 0.0,
                                             Add, Bypass)
                nc.vector.scalar_tensor_tensor(
                    avg8[d // 2][:, d % 2, :], sc[:], vsum[:], denrT[:],
                    Add, Mult)

            # qT after the v stream on the SP queue (needed from gate phase)
            qT = qpool.tile([P, ND, T], bf16, tag="qT")
            nc.sync.dma_start(qT[:], qTd[:].rearrange("d p t -> p d t"))

            # ---- FFN1: h1 = relu(avg @ W1 + b1), fp8 DoubleRow ------------
            h1 = [h1pool.tile([P, 2, T], fp8, tag="h1", name="h1") for _ in range(NP)]
            for j in range(ND):
                w1t = wpool.tile([P, NP, 2, P], fp8, tag="w12")
                nc.scalar.dma_start(w1t[:], w1s[j])
                pss = [pspool.tile([P, BT], f32, tag="mm", name="pss")
                       for _ in range(NBLK)]
                for p in range(NP):
                    for blk in range(NBLK):
                        nc.tensor.matmul(
                            pss[blk][:], w1t[:, p, :, :],
                            avg8[p][:, :, blk * BT:(blk + 1) * BT],
                            start=(p == 0), stop=(p == NP - 1),
                            perf_mode=DR)
                for blk in range(NBLK):
                    nc.scalar.activation(
                        h1[j // 2][:, j % 2, blk * BT:(blk + 1) * BT],
                        pss[blk][:], Relu, bias=b1T[:, j:j + 1])

            # ---- FFN2: h = h1 @ W2 + b2, fp8 DoubleRow --------------------
            hh = [hpool.tile([P, 2, T], fp8, tag="hh", name="hh") for _ in range(NP)]
            for j in range(ND):
                w2t = wpool.tile([P, NP, 2, P], fp8, tag="w12")
                nc.scalar.dma_start(w2t[:], w2s[j])
                pss = [pspool.tile([P, BT], f32, tag="mm", name="pss")
                       for _ in range(NBLK)]
                for p in range(NP):
                    for blk in range(NBLK):
                        nc.tensor.matmul(
                            pss[blk][:], w2t[:, p, :, :],
                            h1[p][:, :, blk * BT:(blk + 1) * BT],
                            start=(p == 0), stop=(p == NP - 1),
                            perf_mode=DR)
                for blk in range(NBLK):
                    nc.scalar.activation(
                        hh[j // 2][:, j % 2, blk * BT:(blk + 1) * BT],
                        pss[blk][:], Ident, bias=b2T[:, j:j + 1])

            # ---- gate + output: bf16 iQ-half, fp8 DR h-half ---------------
            gates = {}
            for gp in range(ND):
                for g in (gp, gp + ND):
                    top = gwpool.tile([P, ND, P], bf16, tag="wgt")
                    nc.scalar.dma_start(top[:], wgt[g])
                    bot = gwpool.tile([P, NP, 2, P], fp8, tag="wgb")
                    nc.scalar.dma_start(bot[:], wgb[g])
                    pss = [pspool.tile([P, BT], f32, tag="mm", name="pss")
                           for _ in range(NBLK)]
                    for c in range(ND):
                        for blk in range(NBLK):
                            nc.tensor.matmul(
                                pss[blk][:], top[:, c, :],
                                qT[:, c, blk * BT:(blk + 1) * BT],
                                start=(c == 0), stop=False)
                    for p in range(NP):
                        for blk in range(NBLK):
                            nc.tensor.matmul(
                                pss[blk][:], bot[:, p, :, :],
                                hh[p][:, :, blk * BT:(blk + 1) * BT],
                                start=False, stop=(p == NP - 1),
                                perf_mode=DR)
                    gt = gatepool.tile([P, T], bf16, tag="gate")
                    for blk in range(NBLK):
                        nc.scalar.activation(
                            gt[:, blk * BT:(blk + 1) * BT], pss[blk][:],
                            Sigm, bias=bgT[:, g:g + 1])
                    gates[g] = gt
                ig, fg = gates.pop(gp), gates.pop(gp + ND)
                tmp = ewpool.tile([P, T], bf16, tag="tmp")
                nc.gpsimd.tensor_mul(tmp[:], ig[:], qT[:, gp, :])
                ob = opool.tile([P, T], bf16, tag="ob")
                nc.vector.tensor_mul(ob[:], fg[:], hh[gp // 2][:, gp % 2, :])
                nc.vector.tensor_add(ob[:], ob[:], tmp[:])
                nc.sync.dma_start(o[gp], ob[:])

    nc.finalize()
    return nc


_CACHED = {}
_last_result = None


def kernel(iQ, iV, W1, b1, W2, b2, Wg, bg):
    import sys
    if '/opt/trn_rl_repo' not in sys.path:
        sys.path.insert(0, '/opt/trn_rl_repo')
    import ml_dtypes
    from concourse.bass_utils import run_bass_kernel_spmd

    BF = ml_dtypes.bfloat16
    F8 = ml_dtypes.float8_e4m3

    iQ = np.asarray(iQ, np.float32)
    iV = np.asarray(iV, np.float32)
    W1 = np.asarray(W1, np.float32)
    b1 = np.asarray(b1, np.float32)
    W2 = np.asarray(W2, np.float32)
    b2 = np.asarray(b2, np.float32)
    Wg = np.asarray(Wg, np.float32)
    bg = np.asarray(bg, np.float32)

    if 'nc' not in _CACHED:
        _CACHED['nc'] = _build_program()
    nc = _CACHED['nc']

    # weight slabs, lhsT layouts (see _build_program dram shapes)
    def dr_slab(W, n_out):
        # [j, k, p, i, m] with K index (p*2+i)*128+k
        return np.ascontiguousarray(
            W.reshape(NP, 2, P, n_out, P).transpose(3, 2, 0, 1, 4)).astype(F8)

    w1s = dr_slab(W1, ND)
    w2s = dr_slab(W2, ND)
    wgt = np.ascontiguousarray(
        Wg[:D].reshape(ND, P, NG, P).transpose(2, 1, 0, 3)).astype(BF)
    wgb = dr_slab(Wg[D:], NG)
    b1c = np.ascontiguousarray(b1.reshape(ND, P).T)
    b2c = np.ascontiguousarray(b2.reshape(ND, P).T)
    bgc = np.ascontiguousarray(bg.reshape(NG, P).T)
    zpre = np.zeros((ND, P, T), BF)

    def t_orient(x):  # [T, D] f32 -> [ND, P, T] bf16
        return np.ascontiguousarray(x.T.reshape(ND, P, T)).astype(BF)

    in_maps = []
    for core in range(8):
        b, half = core // 2, core % 2
        off = half * T
        den = np.ascontiguousarray(np.broadcast_to(
            1.0 / np.arange(off + 1, off + T + 1, dtype=np.float32),
            (P, T))).astype(BF)
        in_maps.append({
            "qTd": t_orient(iQ[b, off:off + T]),
            "vTd": t_orient(iV[b, off:off + T]),
            "vpreT": (t_orient(iV[b, :T]) if half == 1 else zpre),
            "denr": den,
            "w1s": w1s, "w2s": w2s, "wgt": wgt, "wgb": wgb,
            "b1c": b1c, "b2c": b2c, "bgc": bgc,
        })

    res = run_bass_kernel_spmd(nc, in_maps, core_ids=list(range(8)))
    global _last_result
    _last_result = res

    out = np.empty((B, S, D), np.float32)
    for core in range(8):
        b, half = core // 2, core % 2
        ot = np.asarray(res.results[core]["o"], dtype=np.float32)
        out[b, half * T:(half + 1) * T] = \
            ot.transpose(2, 0, 1).reshape(T, D)
    return out


# revision 5
# speedup vs baseline: 2.0930x; 1.3552x over previous
"""Trainium2 Bass kernel for nn_AverageAttn (B=4, S=4096, D=H=1024, 8 cores).

out = igate * iQ + fgate * h, where
  avg  = causal cumulative average of iV along seq
  h    = relu(avg @ W1 + b1) @ W2 + b2
  ifg  = sigmoid(concat(iQ, h) @ Wg + bg);  igate, fgate = split(ifg)

Sharding: 8 cores = (batch b, seq half). Each core processes T=2048 tokens.

v3 design:
 - Host supplies all activations pre-transposed (T-orientation
   [feature, token]) in bf16; output leaves T-oriented bf16.
 - Cumsum on Vector/Pool engines (tensor_tensor_scan); first-half carry
   via Activation-engine accumulate; avg = (scan + carry) * (1/n) in one
   scalar_tensor_tensor.  Zero PE scan work.
 - FFN1/FFN2 and the h-half of the gate in fp8 e4m3 DoubleRow (K=256 per
   instruction); gate iQ-half bf16.  PSUM accumulates fp32.
 - Startup latency hidden: the gate iQ-half for gate chunks 0..7 runs
   FIRST (needs only qT), staged to SBUF as bf16 zq; later only the
   h-half is accumulated and added back before the sigmoid.
 - One [128,2048] activation per output chunk over a 4-bank PSUM tile.
"""

import numpy as np

B, S, D = 4, 4096, 1024
H = 1024
T = S // 2              # tokens per core
P = 128
ND = D // P             # 8 feature chunks
NG = 2 * D // P         # 16 gate chunks
NP = ND // 2            # 4 DoubleRow K-pairs
NBLK = 4                # 512-token matmul blocks
BT = T // NBLK          # 512
POOL_SCAN_D = (1, 4, 7)  # scan chunks routed to the Pool engine


def _build_program():
    import contextlib
    import concourse.bass as bass  # noqa: F401
    import concourse.tile as tile
    from concourse import mybir, bacc

    f32 = mybir.dt.float32
    bf16 = mybir.dt.bfloat16
    fp8 = mybir.dt.float8e4
    Relu = mybir.ActivationFunctionType.Relu
    Ident = mybir.ActivationFunctionType.Identity
    Sigm = mybir.ActivationFunctionType.Sigmoid
    DR = mybir.MatmulPerfMode.DoubleRow
    Add = mybir.AluOpType.add
    Mult = mybir.AluOpType.mult
    Bypass = mybir.AluOpType.bypass

    nc = bacc.Bacc("TRN2", target_bir_lowering=False)

    vTd = nc.dram_tensor("vTd", [ND, P, T], bf16, kind="ExternalInput")
    vpreT = nc.dram_tensor("vpreT", [ND, P, T], bf16, kind="ExternalInput")
    qTd = nc.dram_tensor("qTd", [ND, P, T], bf16, kind="ExternalInput")
    denr = nc.dram_tensor("denr", [P, T], bf16, kind="ExternalInput")
    w1s = nc.dram_tensor("w1s", [ND, P, NP, 2, P], fp8, kind="ExternalInput")
    w2s = nc.dram_tensor("w2s", [ND, P, NP, 2, P], fp8, kind="ExternalInput")
    wgt = nc.dram_tensor("wgt", [NG, P, ND, P], bf16, kind="ExternalInput")
    wgb = nc.dram_tensor("wgb", [NG, P, NP, 2, P], fp8, kind="ExternalInput")
    b1c = nc.dram_tensor("b1c", [P, ND], f32, kind="ExternalInput")
    b2c = nc.dram_tensor("b2c", [P, ND], f32, kind="ExternalInput")
    bgc = nc.dram_tensor("bgc", [P, NG], f32, kind="ExternalInput")
    o = nc.dram_tensor("o", [ND, P, T], bf16, kind="ExternalOutput")

    with tile.TileContext(nc) as tc:
        ctx = contextlib.ExitStack()
        with ctx:
            cpool = ctx.enter_context(tc.tile_pool(name="consts", bufs=1))
            qpool = ctx.enter_context(tc.tile_pool(name="qq", bufs=2))
            zqpool = ctx.enter_context(tc.tile_pool(name="zq", bufs=ND))
            vpool = ctx.enter_context(tc.tile_pool(name="vstream", bufs=3))
            prepool = ctx.enter_context(tc.tile_pool(name="prestream", bufs=2))
            scanpool = ctx.enter_context(tc.tile_pool(name="scans", bufs=2))
            scrpool = ctx.enter_context(tc.tile_pool(name="scratch", bufs=1))
            vsumpool = ctx.enter_context(tc.tile_pool(name="vsum", bufs=ND))
            avgpool = ctx.enter_context(tc.tile_pool(name="avg", bufs=NP))
            h1pool = ctx.enter_context(tc.tile_pool(name="h1", bufs=NP))
            hpool = ctx.enter_context(tc.tile_pool(name="hh", bufs=NP))
            wpool = ctx.enter_context(tc.tile_pool(name="w12", bufs=2))
            gwpool = ctx.enter_context(tc.tile_pool(name="gw", bufs=2))
            gatepool = ctx.enter_context(tc.tile_pool(name="gates", bufs=4))
            zspool = ctx.enter_context(tc.tile_pool(name="zs", bufs=2))
            ewpool = ctx.enter_context(tc.tile_pool(name="ew", bufs=2))
            opool = ctx.enter_context(tc.tile_pool(name="outs", bufs=2))
            pspool = ctx.enter_context(
                tc.tile_pool(name="psmm", bufs=2, space="PSUM"))

            # ---- tiny consts (Activation HWDGE queue) ---------------------
            denrT = cpool.tile([P, T], bf16, tag="denr")
            nc.scalar.dma_start(denrT[:], denr[:])
            b1T = cpool.tile([P, ND], f32, tag="b1")
            nc.scalar.dma_start(b1T[:], b1c[:])
            b2T = cpool.tile([P, ND], f32, tag="b2")
            nc.scalar.dma_start(b2T[:], b2c[:])
            bgT = cpool.tile([P, NG], f32, tag="bg")
            nc.scalar.dma_start(bgT[:], bgc[:])

            # ---- qT in two halves on the SP queue (earliest PE dep) -------
            qTh = []
            for h in range(2):
                qt = qpool.tile([P, ND // 2, T], bf16, tag="qT", name="qt")
                nc.sync.dma_start(
                    qt[:], qTd[h * 4:(h + 1) * 4].rearrange("d p t -> p d t"))
                qTh.append(qt)

            def qT(c):  # [P, T] view of iQ chunk c
                return qTh[c // 4][:, c % 4, :]

            # ---- v streams: vT on SP (after qT), vpre on Pool SWDGE -------
            vts, pres = [], []
            for d in range(ND):
                vt = vpool.tile([P, T], bf16, tag="v", name="vt")
                nc.sync.dma_start(vt[:], vTd[d])
                vts.append(vt)
                pre = prepool.tile([P, T], bf16, tag="pre", name="pre")
                nc.gpsimd.dma_start(pre[:], vpreT[d])
                pres.append(pre)

            # ---- EARLY: gate iQ-half for g=0..7, staged to SBUF bf16 ------
            zq = []
            for g in range(ND):
                top = gwpool.tile([P, ND, P], bf16, tag="wgt", name="top")
                nc.scalar.dma_start(top[:], wgt[g])
                ps = pspool.tile([P, NBLK * BT], f32, tag="mm", name="ps")
                for c in range(ND):
                    for blk in range(NBLK):
                        nc.tensor.matmul(
                            ps[:, blk * BT:(blk + 1) * BT], top[:, c, :],
                            qT(c)[:, blk * BT:(blk + 1) * BT],
                            start=(c == 0), stop=(c == ND - 1))
                zt = zqpool.tile([P, T], bf16, tag="zq", name="zq")
                nc.scalar.activation(zt[:], ps[:], Ident)
                zq.append(zt)

            # ---- scan phase: avg8 pair tiles [P, 2, T] fp8 ----------------
            # cumsum with the first-half carry as the scan initial (DVE);
            # the * (1/n) multiply runs on Pool (plain TensorTensor).
            avg8 = [avgpool.tile([P, 2, T], fp8, tag="avg", name="avg8")
                    for _ in range(NP)]
            for d in range(ND):
                vsum = vsumpool.tile([P, 1], f32, tag="vsum", name="vsum")
                scr = scrpool.tile([P, T], bf16, tag="scr", name="scr")
                nc.scalar.activation(scr[:], pres[d][:], Ident,
                                     accum_out=vsum[:])
                sc = scanpool.tile([P, T], bf16, tag="scan", name="sc")
                nc.vector.tensor_tensor_scan(sc[:], vts[d][:], vts[d][:],
                                             vsum[:], Add, Bypass)
                nc.gpsimd.tensor_mul(avg8[d // 2][:, d % 2, :], sc[:],
                                     denrT[:])

            # ---- FFN1: h1 = relu(avg @ W1 + b1), fp8 DoubleRow ------------
            h1 = [h1pool.tile([P, 2, T], fp8, tag="h1", name="h1")
                  for _ in range(NP)]
            for j in range(ND):
                w1t = wpool.tile([P, NP, 2, P], fp8, tag="w12", name="w1t")
                nc.scalar.dma_start(w1t[:], w1s[j])
                ps = pspool.tile([P, NBLK * BT], f32, tag="mm", name="ps")
                for p in range(NP):
                    for blk in range(NBLK):
                        nc.tensor.matmul(
                            ps[:, blk * BT:(blk + 1) * BT], w1t[:, p, :, :],
                            avg8[p][:, :, blk * BT:(blk + 1) * BT],
                            start=(p == 0), stop=(p == NP - 1),
                            perf_mode=DR)
                nc.scalar.activation(h1[j // 2][:, j % 2, :], ps[:], Relu,
                                     bias=b1T[:, j:j + 1])

            # ---- FFN2: h = h1 @ W2 + b2, fp8 DoubleRow --------------------
            hh = [hpool.tile([P, 2, T], fp8, tag="hh", name="hh")
                  for _ in range(NP)]
            for j in range(ND):
                w2t = wpool.tile([P, NP, 2, P], fp8, tag="w12", name="w2t")
                nc.scalar.dma_start(w2t[:], w2s[j])
                ps = pspool.tile([P, NBLK * BT], f32, tag="mm", name="ps")
                for p in range(NP):
                    for blk in range(NBLK):
                        nc.tensor.matmul(
                            ps[:, blk * BT:(blk + 1) * BT], w2t[:, p, :, :],
                            h1[p][:, :, blk * BT:(blk + 1) * BT],
                            start=(p == 0), stop=(p == NP - 1),
                            perf_mode=DR)
                nc.scalar.activation(hh[j // 2][:, j % 2, :], ps[:], Ident,
                                     bias=b2T[:, j:j + 1])

            # ---- gate rest + output ---------------------------------------
            def h_half(ps, g, start):
                bot = gwpool.tile([P, NP, 2, P], fp8, tag="wgb", name="bot")
                nc.scalar.dma_start(bot[:], wgb[g])
                for p in range(NP):
                    for blk in range(NBLK):
                        nc.tensor.matmul(
                            ps[:, blk * BT:(blk + 1) * BT], bot[:, p, :, :],
                            hh[p][:, :, blk * BT:(blk + 1) * BT],
                            start=(start and p == 0), stop=(p == NP - 1),
                            perf_mode=DR)

            for gp in range(ND):
                # igate chunk gp: h-half into PSUM, add staged zq, sigmoid
                ps = pspool.tile([P, NBLK * BT], f32, tag="mm", name="ps")
                h_half(ps, gp, start=True)
                zs = zspool.tile([P, T], bf16, tag="zs", name="zs")
                nc.vector.tensor_add(zs[:], ps[:], zq[gp][:])
                ig = gatepool.tile([P, T], bf16, tag="gate", name="ig")
                nc.scalar.activation(ig[:], zs[:], Sigm, bias=bgT[:, gp:gp + 1])

                # fgate chunk gp+8: full accumulation
                g = gp + ND
                top = gwpool.tile([P, ND, P], bf16, tag="wgt", name="topf")
                nc.scalar.dma_start(top[:], wgt[g])
                ps2 = pspool.tile([P, NBLK * BT], f32, tag="mm", name="ps2")
                for c in range(ND):
                    for blk in range(NBLK):
                        nc.tensor.matmul(
                            ps2[:, blk * BT:(blk + 1) * BT], top[:, c, :],
                            qT(c)[:, blk * BT:(blk + 1) * BT],
                            start=(c == 0), stop=False)
                h_half(ps2, g, start=False)
                fg = gatepool.tile([P, T], bf16, tag="gate", name="fg")
                nc.scalar.activation(fg[:], ps2[:], Sigm,
                                     bias=bgT[:, g:g + 1])

                # out = ig * iQ + fg * h  (T-orientation)
                tmp = ewpool.tile([P, T], bf16, tag="tmp", name="tmp")
                nc.vector.tensor_mul(tmp[:], ig[:], qT(gp)[:])
                ob = opool.tile([P, T], bf16, tag="ob", name="ob")
                nc.gpsimd.tensor_mul(ob[:], fg[:], hh[gp // 2][:, gp % 2, :])
                nc.vector.tensor_add(ob[:], ob[:], tmp[:])
                nc.sync.dma_start(o[gp], ob[:])

    nc.finalize()
    return nc


_CACHED = {}
_last_result = None


def kernel(iQ, iV, W1, b1, W2, b2, Wg, bg):
    import sys
    if '/opt/trn_rl_repo' not in sys.path:
        sys.path.insert(0, '/opt/trn_rl_repo')
    import ml_dtypes
    from concourse.bass_utils import run_bass_kernel_spmd

    BF = ml_dtypes.bfloat16
    F8 = ml_dtypes.float8_e4m3

    iQ = np.asarray(iQ, np.float32)
    iV = np.asarray(iV, np.float32)
    W1 = np.asarray(W1, np.float32)
    b1 = np.asarray(b1, np.float32)
    W2 = np.asarray(W2, np.float32)
    b2 = np.asarray(b2, np.float32)
    Wg = np.asarray(Wg, np.float32)
    bg = np.asarray(bg, np.float32)

    if 'nc' not in _CACHED:
        _CACHED['nc'] = _build_program()
    nc = _CACHED['nc']

    # weight slabs, lhsT layouts (see _build_program dram shapes)
    def dr_slab(W, n_out):
        # [j, k, p, i, m] with K index (p*2+i)*128+k
        return np.ascontiguousarray(
            W.reshape(NP, 2, P, n_out, P).transpose(3, 2, 0, 1, 4)).astype(F8)

    w1s = dr_slab(W1, ND)
    w2s = dr_slab(W2, ND)
    wgt = np.ascontiguousarray(
        Wg[:D].reshape(ND, P, NG, P).transpose(2, 1, 0, 3)).astype(BF)
    wgb = dr_slab(Wg[D:], NG)
    b1c = np.ascontiguousarray(b1.reshape(ND, P).T)
    b2c = np.ascontiguousarray(b2.reshape(ND, P).T)
    bgc = np.ascontiguousarray(bg.reshape(NG, P).T)
    zpre = np.zeros((ND, P, T), BF)

    def t_orient(x):  # [T, D] f32 -> [ND, P, T] bf16
        return np.ascontiguousarray(x.T.reshape(ND, P, T)).astype(BF)

    in_maps = []
    for core in range(8):
        b, half = core // 2, core % 2
        off = half * T
        den = np.ascontiguousarray(np.broadcast_to(
            1.0 / np.arange(off + 1, off + T + 1, dtype=np.float32),
            (P, T))).astype(BF)
        in_maps.append({
            "qTd": t_orient(iQ[b, off:off + T]),
            "vTd": t_orient(iV[b, off:off + T]),
            "vpreT": (t_orient(iV[b, :T]) if half == 1 else zpre),
            "denr": den,
            "w1s": w1s, "w2s": w2s, "wgt": wgt, "wgb": wgb,
            "b1c": b1c, "b2c": b2c, "bgc": bgc,
        })

    res = run_bass_kernel_spmd(nc, in_maps, core_ids=list(range(8)))
    global _last_result
    _last_result = res

    out = np.empty((B, S, D), np.float32)
    for core in range(8):
        b, half = core // 2, core % 2
        ot = np.asarray(res.results[core]["o"], dtype=np.float32)
        out[b, half * T:(half + 1) * T] = \
            ot.transpose(2, 0, 1).reshape(T, D)
    return out
